# revision 17
# baseline (speedup 1.0000x reference)
"""Deformable conv block kernel for 8 Trainium2 NeuronCores.

Shards batch B=8 across 8 cores (pure data parallel). Per core:
  h1 = lrelu(conv3x3(hr, w1)); h2 = lrelu(conv3x3(h1, w2)); est = conv3x3(h2, w3)
  off = conv3x3(est, wo)  -> per-channel (dy, dx)
  sampled = bilinear(hr, grid + off)   (exact, 5x5 hat window)
  out = conv3x3(sampled, wc)
The lr_features path in the reference is dead (est[B:] depends only on hr).

Convs run as 9-tap shifted fp16 matmuls on the PE (fp32 PSUM accumulate),
2 row-taps K-packed into K=128 per matmul; PSUM is evacuated by single
ACT ops (Prelu alpha=0.1 / Copy) into band-sized SBUF tiles stored with
one DMA per band. Bilinear uses the separable negated-hat identity
  out = sum_dy min(|sy-dy|-1,0) * sum_dx min(|sx-dx|-1,0) * hr[y+dy, x+dx]
(negations cancel), exact for |off| < 2 with 2-wide replicate padding
(replicate-pad sampling == coordinate clipping). Offsets stay raw f16 —
no per-pixel clip ops. All bilinear DVE ops are f16 with 4B-aligned APs
(even/odd column copies of hr) so tensor_tensor runs in 2x mode and
tensor_scalar in 4x mode.

Every producer band emits 33 rows (one row of overlap with the next
band, recomputed) so each pipeline stage depends only on bands produced
in EARLIER steps — the six stages decouple and the PE streams matmuls
back-to-back instead of serializing through evac/store/load latency
(which kept the HAM clock gate cold).
"""
import numpy as np
from contextlib import ExitStack

import concourse.bass as bass
import concourse.tile as tile
from concourse import bacc, mybir
from concourse.bass_utils import run_bass_kernel_spmd

F32 = mybir.dt.float32
F16 = mybir.dt.float16
ALU = mybir.AluOpType
ACTF = mybir.ActivationFunctionType

B, C, H, W = 8, 64, 160, 160
HW = H * W
WP = 162          # conv-padded width  (image col + 1)
HP = 164          # conv-padded rows   (image row + 1; rows 0,161..163 zero)
WS = 164          # samp padded width  (image col + 2; cols 0:2,162:164 zero)
WH = 166          # hr replicate-padded DRAM width (image col + 2, +2 spare)
HH = 164          # hr replicate-padded rows (image row + 2)
R_C = 32          # rows per pipeline band (producers emit 33 w/ overlap)
R_B = 17          # bilinear rows per partition-half per superblock
N_CORES = 8
N_BAND = H // R_C          # 5 bands per stage
FD = R_B * W               # 2720

_CACHE = {}


def _tiles_for(rows_out):
    full3 = rows_out // 3
    rem = rows_out - 3 * full3
    t = [(3 * k, 3) for k in range(full3)]
    if rem:
        t.append((3 * full3, rem))
    return t


def _conv_band(nc, pools, src, src_w, col0, dst, wA, wD, wE, M, act_func,
               in_dt, dst_kind, r0, sid, rows_out, uid):
    """One band of a 3x3 conv stage, 3-row (N=480) PSUM tiles, 5 matmuls
    per tile: 3x K=128 (ky0/ky1 row-pair packed) + 1x K=128 (ky2 kx0/kx1
    col-pair packed via a second shifted input tile) + 1x K=64 (ky2 kx2).

    Stages emit staggered extra bottom rows (telescoping halos: s1 +4,
    s2 +3, s3 +2, s4 +1) so every consumer band depends only on producer
    bands issued in strictly earlier pipeline steps — the stages stay
    decoupled and the PE streams. Redundant rows are recomputed
    identically by adjacent bands. Loads never touch rows a later band
    will write (adjacent-band overlaps are write-after-read only, made
    benign by reverse stage-issue order within each step).
    """
    p_in, p_in2, p_ps, p_ev = pools
    in_t = p_in.tile([128, 38 * WP], in_dt, name=f"cin{sid}_{uid}",
                     tag="cin")
    iv = src.rearrange("c (r w) -> c r w", w=src_w)
    nc.gpsimd.dma_start(in_t[0:64, 0:(rows_out + 2) * WP],
                        iv[:, r0:r0 + rows_out + 2, col0:col0 + WP])
    nc.gpsimd.dma_start(in_t[64:128, 0:(rows_out + 1) * WP],
                        iv[:, r0 + 1:r0 + rows_out + 2, col0:col0 + WP])
    it = in_t.rearrange("p (r w) -> p r w", w=WP)
    in_t2 = p_in2.tile([128, 36 * WP], in_dt, name=f"ci2{sid}_{uid}",
                       tag="cin2")
    it2 = in_t2.rearrange("p (r w) -> p r w", w=WP)
    nc.gpsimd.dma_start(it2[0:64, 0:rows_out, 0:WP],
                        iv[:, r0 + 2:r0 + 2 + rows_out, col0:col0 + WP])
    nc.gpsimd.dma_start(it2[64:128, 0:rows_out, 0:WP - 1],
                        iv[:, r0 + 2:r0 + 2 + rows_out,
                           col0 + 1:col0 + WP])

    tiles = _tiles_for(rows_out)

    if dst_kind == "padded":
        ev = p_ev.tile([M, 36 * WP], F16, name=f"evp{sid}_{uid}", tag="evp")
        e3 = ev.rearrange("p (r w) -> p r w", w=WP)
        nc.gpsimd.memset(e3[:, 0:rows_out, 0:1], 0.0)
        nc.gpsimd.memset(e3[:, 0:rows_out, 161:162], 0.0)
    elif dst_kind == "offsets":
        ev = p_ev.tile([128, 33 * W], F16, name=f"evo{sid}_{uid}",
                       tag="evo")

    for tl, nr in tiles:
        N = nr * W
        ps = p_ps.tile([M, 480], F32, name=f"ps{sid}_{uid}_{tl}", tag="ps")
        for kx in range(3):
            nc.tensor.matmul(
                ps[:, 0:N], wA[:, kx * M:(kx + 1) * M],
                it[:, tl:tl + nr, kx:kx + 160],
                start=(kx == 0), stop=False)
        nc.tensor.matmul(
            ps[:, 0:N], wD[:, 0:M],
            it2[:, tl:tl + nr, 0:160],
            start=False, stop=False)
        nc.tensor.matmul(
            ps[:, 0:N], wE[0:64, 0:M],
            it2[0:64, tl:tl + nr, 2:162],
            start=False, stop=True)

        if dst_kind == "padded":
            e3i = e3[:, tl:tl + nr, 1:161]
            if act_func == ACTF.Prelu:
                nc.scalar.activation(e3i, ps[:, 0:N], ACTF.Prelu, alpha=0.1)
            else:
                nc.scalar.activation(e3i, ps[:, 0:N], act_func)
        elif dst_kind == "offsets":
            nc.scalar.activation(ev[:, tl * W:tl * W + N], ps[:, 0:N],
                                 ACTF.Copy)
        else:  # flat f16 output, per-tile store
            evf = p_ev.tile([M, 480], F16, name=f"evf{sid}_{uid}_{tl}",
                            tag="evf")
            nc.scalar.activation(evf[:, 0:N], ps[:, 0:N], act_func)
            nc.sync.dma_start(dst[:, (r0 + tl) * W:(r0 + tl + nr) * W],
                              evf[:, 0:N])

    if dst_kind == "padded":
        dv = dst.rearrange("c (r w) -> c r w", w=WP)
        nc.sync.dma_start(dv[:, r0 + 1:r0 + 1 + rows_out, :],
                          ev[:, 0:rows_out * WP])
    elif dst_kind == "offsets":
        offy_d, offx_d = dst
        nc.sync.dma_start(offy_d[:, r0 * W:(r0 + rows_out) * W],
                          ev[0:64, 0:rows_out * W])
        nc.sync.dma_start(offx_d[:, r0 * W:(r0 + rows_out) * W],
                          ev[64:128, 0:rows_out * W])


def _bilinear_sb(nc, bpools, hr16, offy_d, offx_d, samp_d, biases, blk):
    """One 33-row superblock of exact 5x5 hat-window bilinear sampling.

    Two 17-row partition halves overlap by one row (16-row step); the
    last superblock shifts its second half down so no row exceeds 159.
    """
    p_ld, p_w, p_t, p_acc = bpools
    rA = R_C * blk
    h1r = rA + 16 if blk < N_BAND - 1 else rA + 15
    starts = (rA, h1r)
    oy = p_ld.tile([128, FD], F16, name=f"oy{blk}", tag="oy")
    ox = p_ld.tile([128, FD], F16, name=f"ox{blk}", tag="ox")
    hrE = p_ld.tile([128, (R_B + 4) * WS], F16, name=f"hrE{blk}", tag="hrE")
    hrO = p_ld.tile([128, (R_B + 4) * WS], F16, name=f"hrO{blk}", tag="hrO")
    hv = hr16.rearrange("c (r w) -> c r w", w=WH)
    for half, r in enumerate(starts):
        p0 = 64 * half
        nc.gpsimd.dma_start(oy[p0:p0 + 64, :], offy_d[:, r * W:(r + R_B) * W])
        nc.gpsimd.dma_start(ox[p0:p0 + 64, :], offx_d[:, r * W:(r + R_B) * W])
        nc.gpsimd.dma_start(hrE[p0:p0 + 64, :], hv[:, r:r + R_B + 4, 0:WS])
        nc.gpsimd.dma_start(hrO[p0:p0 + 64, :], hv[:, r:r + R_B + 4, 1:WS + 1])
    hE = hrE.rearrange("p (r w) -> p r w", w=WS)
    hO = hrO.rearrange("p (r w) -> p r w", w=WS)

    # negated hat weights for x: wx_d = min(|ox - d| - 1, 0),  d = dxi - 2
    tmp = p_t.tile([128, FD], F16, name=f"tmp{blk}", tag="tmp")
    wx = []
    for dxi in range(5):
        w = p_w.tile([128, FD], F16, name=f"wx{dxi}_{blk}", tag=f"wx{dxi}")
        nc.scalar.activation(w[:, :], ox[:, :], ACTF.Abs,
                             bias=biases[dxi][:, :])
        nc.vector.tensor_scalar(w[:, :], w[:, :], 1.0, 0.0,
                                ALU.subtract, ALU.min)
        wx.append(w)

    acc = p_acc.tile([128, R_B * WS], F16, name=f"acc{blk}", tag="acc")
    a3 = acc.rearrange("p (r w) -> p r w", w=WS)
    nc.gpsimd.memset(a3[:, :, 0:2], 0.0)
    nc.gpsimd.memset(a3[:, :, 162:164], 0.0)
    acc_i = a3[:, :, 2:162]
    hs = p_t.tile([128, FD], F16, name=f"hs{blk}", tag="hs")
    wy = p_t.tile([128, FD], F16, name=f"wy{blk}", tag="wy")
    # taps: (tile, col offset) for dx = -2..2; even dx from hrE, odd from hrO
    taps = [(hE, 0), (hO, 0), (hE, 2), (hO, 2), (hE, 4)]
    for dyi in range(5):
        for k, (ht, c0) in enumerate(taps):
            v = ht[:, dyi:dyi + R_B, c0:c0 + W]
            if k == 0:
                nc.vector.tensor_mul(hs[:, :], wx[k][:, :], v)
            else:
                nc.vector.tensor_mul(tmp[:, :], wx[k][:, :], v)
                nc.vector.tensor_add(hs[:, :], hs[:, :], tmp[:, :])
        nc.scalar.activation(wy[:, :], oy[:, :], ACTF.Abs,
                             bias=biases[dyi][:, :])
        nc.vector.tensor_scalar(wy[:, :], wy[:, :], 1.0, 0.0,
                                ALU.subtract, ALU.min)
        if dyi == 0:
            nc.vector.tensor_mul(acc_i, wy[:, :], hs[:, :])
        else:
            nc.vector.tensor_mul(tmp[:, :], wy[:, :], hs[:, :])
            nc.vector.tensor_add(acc_i, acc_i,
                                 tmp.rearrange("p (r w) -> p r w", w=W))
    sv = samp_d.rearrange("c (r w) -> c r w", w=WS)
    for half, r in enumerate(starts):
        nc.sync.dma_start(sv[:, r + 1:r + 1 + R_B, :],
                          acc[64 * half:64 * half + 64, :])


def build_program(debug_outputs=False):
    ikind = "ExternalOutput" if debug_outputs else "Internal"
    nc = bacc.Bacc("TRN2", target_bir_lowering=False, debug=False,
                   num_devices=N_CORES)
    xpad = nc.dram_tensor("xpad", [C, HP * WP], F16, kind="ExternalInput").ap()
    hr16 = nc.dram_tensor("hr16", [C, HH * WH], F16,
                          kind="ExternalInput").ap()
    w_in = {}
    for s, m in (("w1", 64), ("w2", 64), ("w3", 64), ("wo", 128),
                 ("wc", 128)):
        w_in[s + "A"] = nc.dram_tensor(s + "A", [128, 3 * m], F16,
                                       kind="ExternalInput").ap()
        w_in[s + "D"] = nc.dram_tensor(s + "D", [128, m], F16,
                                       kind="ExternalInput").ap()
        w_in[s + "E"] = nc.dram_tensor(s + "E", [64, m], F16,
                                       kind="ExternalInput").ap()

    out = nc.dram_tensor("out", [128, HW], F16, kind="ExternalOutput").ap()

    h1p = nc.dram_tensor("h1p", [C, HP * WP], F16, kind=ikind).ap()
    h2p = nc.dram_tensor("h2p", [C, HP * WP], F16, kind=ikind).ap()
    estp = nc.dram_tensor("estp", [C, HP * WP], F16, kind=ikind).ap()
    offy_d = nc.dram_tensor("offy", [C, HW], F16, kind=ikind).ap()
    offx_d = nc.dram_tensor("offx", [C, HW], F16, kind=ikind).ap()
    samp_d = nc.dram_tensor("samp", [C, HP * WS], F16, kind=ikind).ap()

    with ExitStack() as ctx:
        tc = ctx.enter_context(tile.TileContext(nc))
        p_const = ctx.enter_context(tc.tile_pool(name="const", bufs=1))

        zrow = p_const.tile([64, 3 * WS], F32, name="zrow")
        nc.vector.memset(zrow[:, :], 0.0)
        zrow16 = zrow.bitcast(F16)
        for buf in (h1p, h2p, estp):
            bv = buf.rearrange("c (r w) -> c r w", w=WP)
            nc.sync.dma_start(bv[:, 0:1, :], zrow16[:, 0:WP])
            nc.sync.dma_start(bv[:, 161:164, :], zrow16[:, 0:3 * WP])
        sv = samp_d.rearrange("c (r w) -> c r w", w=WS)
        nc.sync.dma_start(sv[:, 0:1, :], zrow16[:, 0:WS])
        nc.sync.dma_start(sv[:, 161:164, :], zrow16[:, 0:3 * WS])

        wsb = {}
        for name, ap in w_in.items():
            t = p_const.tile(list(ap.shape), ap.dtype, name="w_" + name)
            nc.sync.dma_start(t[:, :], ap[:, :])
            wsb[name] = t

        p_in = ctx.enter_context(tc.tile_pool(name="c_in", bufs=3))
        p_in2 = ctx.enter_context(tc.tile_pool(name="c_in2", bufs=1))
        p_ps = ctx.enter_context(tc.tile_pool(name="c_ps", bufs=7,
                                              space="PSUM"))
        p_ev = ctx.enter_context(tc.tile_pool(name="c_ev", bufs=2))
        pools = (p_in, p_in2, p_ps, p_ev)

        p_ld = ctx.enter_context(tc.tile_pool(name="b_ld", bufs=2))
        p_w = ctx.enter_context(tc.tile_pool(name="b_w", bufs=1))
        p_t = ctx.enter_context(tc.tile_pool(name="b_t", bufs=1))
        p_acc = ctx.enter_context(tc.tile_pool(name="b_acc", bufs=2))
        bpools = (p_ld, p_w, p_t, p_acc)
        biases = []
        for i, d in enumerate((-2, -1, 0, 1, 2)):
            bt = p_const.tile([128, 1], F32, name=f"bias{i}")
            nc.vector.memset(bt[:, :], float(-d))
            biases.append(bt)

        # conv stage specs: (src, src_w, col0, dst, wkey, M, act, kind,
        #                     sid, halo_extra)
        conv_specs = [
            (xpad, WP, 0, h1p, "w1", 64, ACTF.Prelu, "padded", 1, 4),
            (h1p, WP, 0, h2p, "w2", 64, ACTF.Prelu, "padded", 2, 3),
            (h2p, WP, 0, estp, "w3", 64, ACTF.Copy, "padded", 3, 2),
            (estp, WP, 0, (offy_d, offx_d), "wo", 128, ACTF.Copy,
             "offsets", 4, 1),
            None,  # s5 = bilinear
            (samp_d, WS, 1, out, "wc", 128, ACTF.Copy, "flat", 6, 0),
        ]

        def conv(s, r0, rows_out, uid):
            sp = conv_specs[s]
            src_t, sw, c0, dst, wk, M, act, kind, sid, _ = sp
            _conv_band(nc, pools, src_t, sw, c0, dst, wsb[wk + "A"],
                       wsb[wk + "D"], wsb[wk + "E"], M, act, F16, kind,
                       r0, sid, rows_out, uid)

        # prologue: band 0 of s1..s4 as four 8-row sub-bands with
        # telescoped halos, so the first bilinear superblock starts early.
        for t in range(7):
            for s in reversed(range(4)):
                j = t - s
                if 0 <= j < 4:
                    extra = conv_specs[s][9]
                    conv(s, 8 * j, 8 + extra, f"p{j}")

        # main loop: at step i, stage s processes band i-s (conv stages
        # start at band 1; bilinear and final conv cover all bands).
        # reverse stage order within each step: adjacent-band
        # redundant-row overlaps become WAR, not false RAW deps.
        for i in range(N_BAND + 5):
            for s in reversed(range(6)):
                b = i - s
                if s < 4:
                    if 1 <= b < N_BAND:
                        extra = conv_specs[s][9] if b < N_BAND - 1 else 0
                        conv(s, R_C * b, 32 + extra, str(b))
                elif s == 4:
                    if 0 <= b < N_BAND:
                        _bilinear_sb(nc, bpools, hr16, offy_d, offx_d,
                                     samp_d, biases, b)
                else:
                    if 0 <= b < N_BAND:
                        conv(5, R_C * b, 32, str(b))
    nc.compile()
    return nc


def _prep_weights(w, m, dtype):
    # w: (Cout, Cin, 3, 3) -> wA [128, 3*m] (ky0/ky1 K-paired per kx),
    # wD [128, m] (ky2: kx0/kx1 K-paired), wE [64, m] (ky2 kx2)
    wA = np.zeros((128, 3 * m), dtype=dtype)
    wD = np.zeros((128, m), dtype=dtype)
    for kx in range(3):
        wA[0:64, kx * m:(kx + 1) * m] = w[:, :, 0, kx].T
        wA[64:128, kx * m:(kx + 1) * m] = w[:, :, 1, kx].T
    wD[0:64, :] = w[:, :, 2, 0].T
    wD[64:128, :] = w[:, :, 2, 1].T
    wE = np.ascontiguousarray(w[:, :, 2, 2].T).astype(dtype)
    return wA, wD, wE


def _host_inputs(inputs):
    hr = np.asarray(inputs["hr_features"], dtype=np.float32)
    shared = {}
    for s, key, m in (("w1", "est_w1", 64), ("w2", "est_w2", 64),
                      ("w3", "est_w3", 64)):
        A, D, E = _prep_weights(np.asarray(inputs[key], np.float32), m,
                                np.float16)
        shared[s + "A"], shared[s + "D"], shared[s + "E"] = A, D, E
    # offset conv: permute output channels to [dy c=0..63 | dx c=0..63]
    wo = np.asarray(inputs["offset_w"], np.float32)
    perm = np.concatenate([np.arange(0, 128, 2), np.arange(1, 128, 2)])
    A, D, E = _prep_weights(wo[perm], 128, np.float16)
    shared["woA"], shared["woD"], shared["woE"] = A, D, E
    A, D, E = _prep_weights(np.asarray(inputs["conv1_w"], np.float32), 128,
                            np.float16)
    shared["wcA"], shared["wcD"], shared["wcE"] = A, D, E

    in_maps = []
    for b in range(B):
        m = dict(shared)
        xpad = np.zeros((C, HP, WP), np.float16)
        xpad[:, 1:161, 1:161] = hr[b]
        m["xpad"] = xpad.reshape(C, HP * WP)
        hr16 = np.pad(hr[b], ((0, 0), (2, 2), (2, 4)),
                      mode="edge").astype(np.float16)
        m["hr16"] = hr16.reshape(C, HH * WH)
        in_maps.append(m)
    return in_maps


def kernel(**inputs):
    if "nc" not in _CACHE:
        _CACHE["nc"] = build_program()
    nc = _CACHE["nc"]
    in_maps = _host_inputs(inputs)
    res = run_bass_kernel_spmd(nc, in_maps, list(range(N_CORES)))
    out = np.stack([res.results[b]["out"].reshape(128, H, W)
                    for b in range(B)])
    return out.astype(np.float32)


# revision 20
# speedup vs baseline: 1.1564x; 1.1564x over previous
"""Deformable conv block kernel for 8 Trainium2 NeuronCores.

Shards batch B=8 across 8 cores (pure data parallel). Per core:
  h1 = lrelu(conv3x3(hr, w1)); h2 = lrelu(conv3x3(h1, w2)); est = conv3x3(h2, w3)
  off = conv3x3(est, wo)  -> per-channel (dy, dx)
  sampled = bilinear(hr, grid + off)   (exact, 5x5 hat window)
  out = conv3x3(sampled, wc)
The lr_features path in the reference is dead (est[B:] depends only on hr).

Convs run as 9-tap shifted fp16 matmuls on the PE (fp32 PSUM accumulate),
2 row-taps K-packed into K=128 per matmul; PSUM is evacuated by single
ACT ops (Prelu alpha=0.1 / Copy) into band-sized SBUF tiles stored with
one DMA per band. Bilinear uses the separable negated-hat identity
  out = sum_dy min(|sy-dy|-1,0) * sum_dx min(|sx-dx|-1,0) * hr[y+dy, x+dx]
(negations cancel), exact for |off| < 2 with 2-wide replicate padding
(replicate-pad sampling == coordinate clipping). Offsets stay raw f16 —
no per-pixel clip ops. All bilinear DVE ops are f16 with 4B-aligned APs
(even/odd column copies of hr) so tensor_tensor runs in 2x mode and
tensor_scalar in 4x mode.

Every producer band emits 33 rows (one row of overlap with the next
band, recomputed) so each pipeline stage depends only on bands produced
in EARLIER steps — the six stages decouple and the PE streams matmuls
back-to-back instead of serializing through evac/store/load latency
(which kept the HAM clock gate cold).
"""
import numpy as np
from contextlib import ExitStack

import concourse.bass as bass
import concourse.tile as tile
from concourse import bacc, mybir
from concourse.bass_utils import run_bass_kernel_spmd

F32 = mybir.dt.float32
F16 = mybir.dt.float16
ALU = mybir.AluOpType
ACTF = mybir.ActivationFunctionType

B, C, H, W = 8, 64, 160, 160
HW = H * W
WP = 162          # conv-padded width  (image col + 1)
HP = 164          # conv-padded rows   (image row + 1; rows 0,161..163 zero)
WS = 164          # samp padded width  (image col + 2; cols 0:2,162:164 zero)
WH = 166          # hr replicate-padded DRAM width (image col + 2, +2 spare)
HH = 164          # hr replicate-padded rows (image row + 2)
R_C = 32          # rows per pipeline band (producers emit 33 w/ overlap)
R_B = 17          # bilinear rows per partition-half per superblock
N_CORES = 8
N_BAND = H // R_C          # 5 bands per stage
FD = R_B * W               # 2720

_CACHE = {}


def _tiles_for(rows_out):
    full3 = rows_out // 3
    rem = rows_out - 3 * full3
    t = [(3 * k, 3) for k in range(full3)]
    if rem:
        t.append((3 * full3, rem))
    return t


def _conv_band(nc, pools, src, src_w, col0, dst, wA, wD, wE, M, act_func,
               in_dt, dst_kind, r0, sid, rows_out, uid):
    """One band of a 3x3 conv stage, 3-row (N=480) PSUM tiles, 5 matmuls
    per tile: 3x K=128 (ky0/ky1 row-pair packed) + 1x K=128 (ky2 kx0/kx1
    col-pair packed via a second shifted input tile) + 1x K=64 (ky2 kx2).

    Stages emit staggered extra bottom rows (telescoping halos: s1 +4,
    s2 +3, s3 +2, s4 +1) so every consumer band depends only on producer
    bands issued in strictly earlier pipeline steps — the stages stay
    decoupled and the PE streams. Redundant rows are recomputed
    identically by adjacent bands. Loads never touch rows a later band
    will write (adjacent-band overlaps are write-after-read only, made
    benign by reverse stage-issue order within each step).
    """
    p_in, p_in2, p_ps, p_ev = pools
    in_t = p_in.tile([128, 22 * WP], in_dt, name=f"cin{sid}_{uid}",
                     tag="cin")
    iv = src.rearrange("c (r w) -> c r w", w=src_w)
    nc.gpsimd.dma_start(in_t[0:64, 0:(rows_out + 2) * WP],
                        iv[:, r0:r0 + rows_out + 2, col0:col0 + WP])
    nc.gpsimd.dma_start(in_t[64:128, 0:(rows_out + 1) * WP],
                        iv[:, r0 + 1:r0 + rows_out + 2, col0:col0 + WP])
    it = in_t.rearrange("p (r w) -> p r w", w=WP)
    in_t2 = p_in2.tile([128, 20 * WP], in_dt, name=f"ci2{sid}_{uid}",
                       tag="cin2")
    it2 = in_t2.rearrange("p (r w) -> p r w", w=WP)
    nc.gpsimd.dma_start(it2[0:64, 0:rows_out, 0:WP],
                        iv[:, r0 + 2:r0 + 2 + rows_out, col0:col0 + WP])
    nc.gpsimd.dma_start(it2[64:128, 0:rows_out, 0:WP - 1],
                        iv[:, r0 + 2:r0 + 2 + rows_out,
                           col0 + 1:col0 + WP])

    tiles = _tiles_for(rows_out)

    if dst_kind == "padded":
        ev = p_ev.tile([M, 20 * WP], F16, name=f"evp{sid}_{uid}", tag="evp")
        e3 = ev.rearrange("p (r w) -> p r w", w=WP)
        nc.gpsimd.memset(e3[:, 0:rows_out, 0:1], 0.0)
        nc.gpsimd.memset(e3[:, 0:rows_out, 161:162], 0.0)
    elif dst_kind == "offsets":
        ev = p_ev.tile([128, 17 * W], F16, name=f"evo{sid}_{uid}",
                       tag="evo")

    for tl, nr in tiles:
        N = nr * W
        ps = p_ps.tile([M, 480], F32, name=f"ps{sid}_{uid}_{tl}", tag="ps")
        for kx in range(3):
            nc.tensor.matmul(
                ps[:, 0:N], wA[:, kx * M:(kx + 1) * M],
                it[:, tl:tl + nr, kx:kx + 160],
                start=(kx == 0), stop=False)
        nc.tensor.matmul(
            ps[:, 0:N], wD[:, 0:M],
            it2[:, tl:tl + nr, 0:160],
            start=False, stop=False)
        nc.tensor.matmul(
            ps[:, 0:N], wE[0:64, 0:M],
            it2[0:64, tl:tl + nr, 2:162],
            start=False, stop=True)

        if dst_kind == "padded":
            e3i = e3[:, tl:tl + nr, 1:161]
            if act_func == ACTF.Prelu:
                nc.scalar.activation(e3i, ps[:, 0:N], ACTF.Prelu, alpha=0.1)
            else:
                nc.scalar.activation(e3i, ps[:, 0:N], act_func)
        elif dst_kind == "offsets":
            nc.scalar.activation(ev[:, tl * W:tl * W + N], ps[:, 0:N],
                                 ACTF.Copy)
        else:  # flat f16 output, per-tile store
            evf = p_ev.tile([M, 480], F16, name=f"evf{sid}_{uid}_{tl}",
                            tag="evf")
            nc.scalar.activation(evf[:, 0:N], ps[:, 0:N], act_func)
            nc.sync.dma_start(dst[:, (r0 + tl) * W:(r0 + tl + nr) * W],
                              evf[:, 0:N])

    if dst_kind == "padded":
        dv = dst.rearrange("c (r w) -> c r w", w=WP)
        nc.sync.dma_start(dv[:, r0 + 1:r0 + 1 + rows_out, :],
                          ev[:, 0:rows_out * WP])
    elif dst_kind == "offsets":
        offy_d, offx_d = dst
        nc.sync.dma_start(offy_d[:, r0 * W:(r0 + rows_out) * W],
                          ev[0:64, 0:rows_out * W])
        nc.sync.dma_start(offx_d[:, r0 * W:(r0 + rows_out) * W],
                          ev[64:128, 0:rows_out * W])


def _bilinear_sb(nc, bpools, hr16, offy_d, offx_d, samp_d, biases, blk):
    """One 33-row superblock of exact 5x5 hat-window bilinear sampling.

    Two 17-row partition halves overlap by one row (16-row step); the
    last superblock shifts its second half down so no row exceeds 159.
    """
    p_ld, p_lo, p_w, p_t, p_acc = bpools
    rA = R_C * blk
    h1r = rA + 16 if blk < N_BAND - 1 else rA + 15
    starts = (rA, h1r)
    oy = p_lo.tile([128, FD], F16, name=f"oy{blk}", tag="oy")
    ox = p_lo.tile([128, FD], F16, name=f"ox{blk}", tag="ox")
    hrE = p_ld.tile([128, (R_B + 4) * WS], F16, name=f"hrE{blk}", tag="hrE")
    hrO = p_ld.tile([128, (R_B + 4) * WS], F16, name=f"hrO{blk}", tag="hrO")
    hv = hr16.rearrange("c (r w) -> c r w", w=WH)
    for half, r in enumerate(starts):
        p0 = 64 * half
        nc.gpsimd.dma_start(oy[p0:p0 + 64, :], offy_d[:, r * W:(r + R_B) * W])
        nc.gpsimd.dma_start(ox[p0:p0 + 64, :], offx_d[:, r * W:(r + R_B) * W])
        nc.gpsimd.dma_start(hrE[p0:p0 + 64, :], hv[:, r:r + R_B + 4, 0:WS])
        nc.gpsimd.dma_start(hrO[p0:p0 + 64, :], hv[:, r:r + R_B + 4, 1:WS + 1])
    hE = hrE.rearrange("p (r w) -> p r w", w=WS)
    hO = hrO.rearrange("p (r w) -> p r w", w=WS)

    # negated hat weights for x: wx_d = min(|ox - d| - 1, 0),  d = dxi - 2
    tmp = p_t.tile([128, FD], F16, name=f"tmp{blk}", tag="tmp")
    wx = []
    for dxi in range(5):
        w = p_w.tile([128, FD], F16, name=f"wx{dxi}_{blk}", tag=f"wx{dxi}")
        nc.scalar.activation(w[:, :], ox[:, :], ACTF.Abs,
                             bias=biases[dxi][:, :])
        nc.vector.tensor_scalar(w[:, :], w[:, :], 1.0, 0.0,
                                ALU.subtract, ALU.min)
        wx.append(w)

    acc = p_acc.tile([128, R_B * WS], F16, name=f"acc{blk}", tag="acc")
    a3 = acc.rearrange("p (r w) -> p r w", w=WS)
    nc.gpsimd.memset(a3[:, :, 0:2], 0.0)
    nc.gpsimd.memset(a3[:, :, 162:164], 0.0)
    acc_i = a3[:, :, 2:162]
    hs = p_t.tile([128, FD], F16, name=f"hs{blk}", tag="hs")
    wy = p_t.tile([128, FD], F16, name=f"wy{blk}", tag="wy")
    # taps: (tile, col offset) for dx = -2..2; even dx from hrE, odd from hrO
    taps = [(hE, 0), (hO, 0), (hE, 2), (hO, 2), (hE, 4)]
    for dyi in range(5):
        for k, (ht, c0) in enumerate(taps):
            v = ht[:, dyi:dyi + R_B, c0:c0 + W]
            if k == 0:
                nc.vector.tensor_mul(hs[:, :], wx[k][:, :], v)
            else:
                nc.vector.tensor_mul(tmp[:, :], wx[k][:, :], v)
                nc.vector.tensor_add(hs[:, :], hs[:, :], tmp[:, :])
        nc.scalar.activation(wy[:, :], oy[:, :], ACTF.Abs,
                             bias=biases[dyi][:, :])
        nc.vector.tensor_scalar(wy[:, :], wy[:, :], 1.0, 0.0,
                                ALU.subtract, ALU.min)
        if dyi == 0:
            nc.vector.tensor_mul(acc_i, wy[:, :], hs[:, :])
        else:
            nc.vector.tensor_mul(tmp[:, :], wy[:, :], hs[:, :])
            nc.vector.tensor_add(acc_i, acc_i,
                                 tmp.rearrange("p (r w) -> p r w", w=W))
    sv = samp_d.rearrange("c (r w) -> c r w", w=WS)
    for half, r in enumerate(starts):
        nc.sync.dma_start(sv[:, r + 1:r + 1 + R_B, :],
                          acc[64 * half:64 * half + 64, :])


def build_program(debug_outputs=False):
    ikind = "ExternalOutput" if debug_outputs else "Internal"
    nc = bacc.Bacc("TRN2", target_bir_lowering=False, debug=False,
                   num_devices=N_CORES)
    xpad = nc.dram_tensor("xpad", [C, HP * WP], F16, kind="ExternalInput").ap()
    hr16 = nc.dram_tensor("hr16", [C, HH * WH], F16,
                          kind="ExternalInput").ap()
    w_in = {}
    for s, m in (("w1", 64), ("w2", 64), ("w3", 64), ("wo", 128),
                 ("wc", 128)):
        w_in[s + "A"] = nc.dram_tensor(s + "A", [128, 3 * m], F16,
                                       kind="ExternalInput").ap()
        w_in[s + "D"] = nc.dram_tensor(s + "D", [128, m], F16,
                                       kind="ExternalInput").ap()
        w_in[s + "E"] = nc.dram_tensor(s + "E", [64, m], F16,
                                       kind="ExternalInput").ap()

    out = nc.dram_tensor("out", [128, HW], F16, kind="ExternalOutput").ap()

    h1p = nc.dram_tensor("h1p", [C, HP * WP], F16, kind=ikind).ap()
    h2p = nc.dram_tensor("h2p", [C, HP * WP], F16, kind=ikind).ap()
    estp = nc.dram_tensor("estp", [C, HP * WP], F16, kind=ikind).ap()
    offy_d = nc.dram_tensor("offy", [C, HW], F16, kind=ikind).ap()
    offx_d = nc.dram_tensor("offx", [C, HW], F16, kind=ikind).ap()
    samp_d = nc.dram_tensor("samp", [C, HP * WS], F16, kind=ikind).ap()

    with ExitStack() as ctx:
        tc = ctx.enter_context(tile.TileContext(nc))
        p_const = ctx.enter_context(tc.tile_pool(name="const", bufs=1))

        zrow = p_const.tile([64, 3 * WS], F32, name="zrow")
        nc.vector.memset(zrow[:, :], 0.0)
        zrow16 = zrow.bitcast(F16)
        for buf in (h1p, h2p, estp):
            bv = buf.rearrange("c (r w) -> c r w", w=WP)
            nc.sync.dma_start(bv[:, 0:1, :], zrow16[:, 0:WP])
            nc.sync.dma_start(bv[:, 161:164, :], zrow16[:, 0:3 * WP])
        sv = samp_d.rearrange("c (r w) -> c r w", w=WS)
        nc.sync.dma_start(sv[:, 0:1, :], zrow16[:, 0:WS])
        nc.sync.dma_start(sv[:, 161:164, :], zrow16[:, 0:3 * WS])

        wsb = {}
        for name, ap in w_in.items():
            t = p_const.tile(list(ap.shape), ap.dtype, name="w_" + name)
            nc.sync.dma_start(t[:, :], ap[:, :])
            wsb[name] = t

        p_in = ctx.enter_context(tc.tile_pool(name="c_in", bufs=3))
        p_in2 = ctx.enter_context(tc.tile_pool(name="c_in2", bufs=2))
        p_ps = ctx.enter_context(tc.tile_pool(name="c_ps", bufs=7,
                                              space="PSUM"))
        p_ev = ctx.enter_context(tc.tile_pool(name="c_ev", bufs=2))
        pools = (p_in, p_in2, p_ps, p_ev)

        p_ld = ctx.enter_context(tc.tile_pool(name="b_ld", bufs=2))
        p_lo = ctx.enter_context(tc.tile_pool(name="b_lo", bufs=2))
        p_w = ctx.enter_context(tc.tile_pool(name="b_w", bufs=2))
        p_t = ctx.enter_context(tc.tile_pool(name="b_t", bufs=1))
        p_acc = ctx.enter_context(tc.tile_pool(name="b_acc", bufs=2))
        bpools = (p_ld, p_lo, p_w, p_t, p_acc)
        biases = []
        for i, d in enumerate((-2, -1, 0, 1, 2)):
            bt = p_const.tile([128, 1], F32, name=f"bias{i}")
            nc.vector.memset(bt[:, :], float(-d))
            biases.append(bt)

        # conv stage specs: (src, src_w, col0, dst, wkey, M, act, kind,
        #                     sid, halo_extra)
        conv_specs = [
            (xpad, WP, 0, h1p, "w1", 64, ACTF.Prelu, "padded", 1, 4),
            (h1p, WP, 0, h2p, "w2", 64, ACTF.Prelu, "padded", 2, 3),
            (h2p, WP, 0, estp, "w3", 64, ACTF.Copy, "padded", 3, 2),
            (estp, WP, 0, (offy_d, offx_d), "wo", 128, ACTF.Copy,
             "offsets", 4, 1),
            None,  # s5 = bilinear
            (samp_d, WS, 1, out, "wc", 128, ACTF.Copy, "flat", 6, 0),
        ]

        def conv(s, r0, rows_out, uid):
            sp = conv_specs[s]
            src_t, sw, c0, dst, wk, M, act, kind, sid, _ = sp
            _conv_band(nc, pools, src_t, sw, c0, dst, wsb[wk + "A"],
                       wsb[wk + "D"], wsb[wk + "E"], M, act, F16, kind,
                       r0, sid, rows_out, uid)

        # 16-row-band wavefront for conv stages (telescoped halos:
        # s1 +4, s2 +3, s3 +2, s4 +1 extra bottom rows; last band plain)
        # so offsets bands arrive ~2x faster than the bilinear consumes
        # them. Bilinear keeps 33-row superblocks (two 17-row halves).
        # Reverse stage order per step keeps redundant-row overlaps WAR.
        NB16 = 10

        def conv16(s, j):
            extra = conv_specs[s][9] if j < NB16 - 1 else 0
            conv(s, 16 * j, 16 + extra, f"f{j}")

        for t in range(16):
            if t >= 7 and (t - 7) % 2 == 0 and (t - 7) // 2 < N_BAND:
                b6 = (t - 7) // 2
                conv(5, 16 * (2 * b6), 16, f"f{2 * b6}")
                conv(5, 16 * (2 * b6 + 1), 16, f"f{2 * b6 + 1}")
            if t >= 5 and (t - 5) % 2 == 0 and (t - 5) // 2 < N_BAND:
                _bilinear_sb(nc, bpools, hr16, offy_d, offx_d,
                             samp_d, biases, (t - 5) // 2)
            for s in reversed(range(4)):
                j = t - s
                if 0 <= j < NB16:
                    conv16(s, j)
    nc.compile()
    return nc


def _prep_weights(w, m, dtype):
    # w: (Cout, Cin, 3, 3) -> wA [128, 3*m] (ky0/ky1 K-paired per kx),
    # wD [128, m] (ky2: kx0/kx1 K-paired), wE [64, m] (ky2 kx2)
    wA = np.zeros((128, 3 * m), dtype=dtype)
    wD = np.zeros((128, m), dtype=dtype)
    for kx in range(3):
        wA[0:64, kx * m:(kx + 1) * m] = w[:, :, 0, kx].T
        wA[64:128, kx * m:(kx + 1) * m] = w[:, :, 1, kx].T
    wD[0:64, :] = w[:, :, 2, 0].T
    wD[64:128, :] = w[:, :, 2, 1].T
    wE = np.ascontiguousarray(w[:, :, 2, 2].T).astype(dtype)
    return wA, wD, wE


def _host_inputs(inputs):
    hr = np.asarray(inputs["hr_features"], dtype=np.float32)
    shared = {}
    for s, key, m in (("w1", "est_w1", 64), ("w2", "est_w2", 64),
                      ("w3", "est_w3", 64)):
        A, D, E = _prep_weights(np.asarray(inputs[key], np.float32), m,
                                np.float16)
        shared[s + "A"], shared[s + "D"], shared[s + "E"] = A, D, E
    # offset conv: permute output channels to [dy c=0..63 | dx c=0..63]
    wo = np.asarray(inputs["offset_w"], np.float32)
    perm = np.concatenate([np.arange(0, 128, 2), np.arange(1, 128, 2)])
    A, D, E = _prep_weights(wo[perm], 128, np.float16)
    shared["woA"], shared["woD"], shared["woE"] = A, D, E
    A, D, E = _prep_weights(np.asarray(inputs["conv1_w"], np.float32), 128,
                            np.float16)
    shared["wcA"], shared["wcD"], shared["wcE"] = A, D, E

    in_maps = []
    for b in range(B):
        m = dict(shared)
        xpad = np.zeros((C, HP, WP), np.float16)
        xpad[:, 1:161, 1:161] = hr[b]
        m["xpad"] = xpad.reshape(C, HP * WP)
        hr16 = np.pad(hr[b], ((0, 0), (2, 2), (2, 4)),
                      mode="edge").astype(np.float16)
        m["hr16"] = hr16.reshape(C, HH * WH)
        in_maps.append(m)
    return in_maps


def kernel(**inputs):
    if "nc" not in _CACHE:
        _CACHE["nc"] = build_program()
    nc = _CACHE["nc"]
    in_maps = _host_inputs(inputs)
    res = run_bass_kernel_spmd(nc, in_maps, list(range(N_CORES)))
    out = np.stack([res.results[b]["out"].reshape(128, H, W)
                    for b in range(B)])
    return out.astype(np.float32)
